# revision 11
# baseline (speedup 1.0000x reference)
"""Trainium2 Bass kernel for nn_ProposalLayer (Faster-RCNN proposal layer).

Pipeline (single NeuronCore, replicated SPMD across 8 cores):
  1. Load raw fg scores [9*4000] into SBUF layout B [128,288] (flat j = a*4000+loc).
  2. Per-partition top-32 extraction (max8 + match_replace), then gpsimd
     kth_largest on the [128,32] extract -> exact 512th-largest raw score tau.
  3. payload = j where score_j > tau else -1; gpsimd sparse_gather compacts the
     selected ~511 candidate indices into 512 slots.
  4. Indirect-DMA gathers: per-candidate record table (host const: delta
     offsets, ref id, anchor params), the 4 deltas, and the score.
  5. Decode + clip + min-size validity for the 512 selected boxes only.
  6. Exact stable rank by (score desc, ref-id asc) via PE-transpose row
     broadcast + compare/accumulate; reorder slot records by rank through a
     DRAM bounce (indirect scatter).
  7. IoU>0.7 suppression masks (vs rank order), greedy NMS solved as a
     fixpoint of masked mat-vec products on the PE (converges in ~4 iters;
     an extra iteration verifies convergence -> flag).
  8. Output positions for the first 300 kept via triangular-matmul prefix
     sums; indirect scatter of box coords into out[301,5] rows (col 0 = 0).

Every data-dependent assumption (selection count <= 512, fixpoint converged,
>=300 kept) is checked on device into a FLAGS output; if any flag trips the
host falls back to an exact numpy replica of the reference (never on the
nominal input distribution).
"""

import numpy as np

import concourse.bass as bass
import concourse.bacc as bacc
import concourse.mybir as mybir
import concourse.tile as tile
from concourse.bass_utils import run_bass_kernel_spmd

F32 = mybir.dt.float32
I32 = mybir.dt.int32
U32 = mybir.dt.uint32
OP = mybir.AluOpType
AF = mybir.ActivationFunctionType

# ---------------- problem constants (hardcoded) ----------------
A = 9
H, W = 50, 80
NLOC = H * W          # 4000
NCAND = A * NLOC      # 36000
NPADC = 128 * 288     # 36864 padded candidates
FEAT_STRIDE = 16
PRE_NMS = 6000
POST_NMS = 300
NMS_THRESH = 0.7
MIN_SIZE = 16.0
NEG = -1.0e20         # "minus inf" stand-in (> -1e29 so kth counts it)
ZAP = -1.0e30         # match_replace fill

SEL = 512             # selected-candidate slots
KADJ = 510            # kth_largest k_adj -> tau = desc[511] (512th largest)
ITERS = 6             # NMS fixpoint iterations (+1 verification round)


def _gen_anchors():
    RATIOS = np.array([0.5, 1.0, 2.0])
    SCALES = np.array([8.0, 16.0, 32.0])

    def _whctrs(a):
        w = a[2] - a[0] + 1.0
        h = a[3] - a[1] + 1.0
        return w, h, a[0] + 0.5 * (w - 1.0), a[1] + 0.5 * (h - 1.0)

    def _mk(ws, hs, cx, cy):
        return np.stack([cx - 0.5 * (ws - 1.0), cy - 0.5 * (hs - 1.0),
                         cx + 0.5 * (ws - 1.0), cy + 0.5 * (hs - 1.0)], axis=1)

    base = np.array([1.0, 1.0, 16.0, 16.0]) - 1.0
    w, h, cx, cy = _whctrs(base)
    size = w * h
    ws = np.round(np.sqrt(size / RATIOS))
    hs = np.round(ws * RATIOS)
    ratio_anchors = _mk(ws, hs, cx, cy)
    out = []
    for ra in ratio_anchors:
        w, h, cx, cy = _whctrs(ra)
        out.append(_mk(w * SCALES, h * SCALES, cx, cy))
    return np.concatenate(out, axis=0).astype(np.float32)  # [9,4]


def _build_tables():
    """Host-constant tables, all keyed by flat j = a*4000 + loc."""
    base = _gen_anchors()  # [9,4] f32
    j = np.arange(NPADC)
    a = np.minimum(j // NLOC, A - 1)
    loc = j % NLOC
    hh = loc // W
    ww = loc % W
    gx = (ww * FEAT_STRIDE).astype(np.float32)
    gy = (hh * FEAT_STRIDE).astype(np.float32)
    # anchor box, f32 math mirrors the reference exactly (all values exact)
    x1 = base[a, 0] + gx
    y1 = base[a, 1] + gy
    x2 = base[a, 2] + gx
    y2 = base[a, 3] + gy
    aw = (x2 - x1 + np.float32(1.0)).astype(np.float32)
    ah = (y2 - y1 + np.float32(1.0)).astype(np.float32)
    acx = (x1 + np.float32(0.5) * aw).astype(np.float32)
    acy = (y1 + np.float32(0.5) * ah).astype(np.float32)
    offd = (a * 4 * NLOC + loc).astype(np.float32)       # dx offset in deltas flat
    idref = (loc * A + a).astype(np.float32)             # reference candidate id
    rec = np.stack([offd, idref, aw, ah,
                    (aw * np.float32(0.5)).astype(np.float32),
                    (ah * np.float32(0.5)).astype(np.float32),
                    acx, acy], axis=1).astype(np.float32)  # [NPADC, 8]
    rec[NCAND:] = 0.0

    # strict-order tiles: for i-group c (i = c*128 + p), cols f in [0,512):
    # strict[c][p, f] = 1.0 iff f > c*128 + p.  Stored as [128, 4*512].
    strict = np.zeros((128, 4 * SEL), dtype=np.float32)
    p = np.arange(128)[:, None]
    f = np.arange(SEL)[None, :]
    for c in range(4):
        strict[:, c * SEL:(c + 1) * SEL] = (f > (c * 128 + p)).astype(np.float32)

    # strictly-lower ones (lhsT for exclusive prefix): low[k, m] = 1 iff k < m
    k = np.arange(128)[:, None]
    m = np.arange(128)[None, :]
    slow = (k < m).astype(np.float32)
    ident = np.eye(128, dtype=np.float32)
    # distinct pad ids for the 512 slots: 40000 + (p*4 + c)
    slotid = (40000.0 + np.arange(SEL, dtype=np.float64).reshape(128, 4)).astype(np.float32)
    # flat entry index k = p*32 + c over the extraction tile
    kidx = np.arange(128 * 32, dtype=np.float64).reshape(128, 32).astype(np.float32)
    # per-partition candidate base j = p*288
    pbase = (np.arange(128, dtype=np.float64) * 288.0).reshape(128, 1).astype(np.float32)
    return rec, strict, slow, ident, slotid, kidx, pbase


_REC, _STRICT, _SLOW, _IDENT, _SLOTID, _KIDX, _PBASE = _build_tables()


def _kth_quantile():
    """Pick q so kth_largest's k_adj == KADJ for n_valid == 4096 exactly."""
    n = 4096
    for bump in range(0, 1000):
        q = 1.0 - (KADJ + 0.5 + bump * 1e-4) / (n - 1)
        omq = max(1, min(int(round((1.0 - q) * 4294967296)), 4294967295))
        if (omq * (n - 1)) >> 32 == KADJ:
            return q
    raise AssertionError("no quantile found")


_KTH_Q = _kth_quantile()


def build_program():
    nc = bacc.Bacc("TRN2", debug=False, enable_asserts=False, num_devices=8)

    scores_t = nc.dram_tensor("scores", [1, 2 * A, H, W], F32, kind="ExternalInput")
    deltas_t = nc.dram_tensor("bbox_deltas", [1, 4 * A, H, W], F32, kind="ExternalInput")
    iminfo_t = nc.dram_tensor("im_info", [1, 3], F32, kind="ExternalInput")
    rec_t = nc.dram_tensor("recs", [NPADC, 8], F32, kind="ExternalInput")
    strict_t = nc.dram_tensor("strict", [128, 4 * SEL], F32, kind="ExternalInput")
    slow_t = nc.dram_tensor("slow", [128, 128], F32, kind="ExternalInput")
    ident_t = nc.dram_tensor("ident", [128, 128], F32, kind="ExternalInput")
    slotid_t = nc.dram_tensor("slotid", [128, 4], F32, kind="ExternalInput")
    kidx_t = nc.dram_tensor("kidx", [128, 32], F32, kind="ExternalInput")
    pbase_t = nc.dram_tensor("pbase", [128, 1], F32, kind="ExternalInput")

    s1_t = nc.dram_tensor("s1", [SEL + 4096, 1], F32)          # cid scatter (+trash)
    s2_t = nc.dram_tensor("s2", [SEL, 8], F32)                 # reorder bounce
    out_t = nc.dram_tensor("out", [POST_NMS + 1, 5], F32, kind="ExternalOutput")
    flags_t = nc.dram_tensor("flags", [1, 8], F32, kind="ExternalOutput")

    with tile.TileContext(nc) as tc:
        _body(tc, scores_t, deltas_t, iminfo_t, rec_t, strict_t, slow_t,
              ident_t, slotid_t, kidx_t, pbase_t, s1_t, s2_t, out_t, flags_t)
    nc.compile()
    return nc


def _body(tc, scores_t, deltas_t, iminfo_t, rec_t, strict_t, slow_t, ident_t,
          slotid_t, kidx_t, pbase_t, s1_t, s2_t, out_t, flags_t):
    nc = tc.nc
    import contextlib
    ctx = contextlib.ExitStack()
    sb = ctx.enter_context(tc.tile_pool(name="sb", bufs=1))
    sbw = ctx.enter_context(tc.tile_pool(name="sbw", bufs=3))   # mask scratch
    ps_rows = ctx.enter_context(tc.tile_pool(name="psrows", bufs=7, space="PSUM"))
    ps_small = ctx.enter_context(tc.tile_pool(name="pssmall", bufs=1, space="PSUM"))

    # ---------------- constants in ----------------
    strict_sb = sb.tile([128, 4 * SEL], F32)
    nc.sync.dma_start(out=strict_sb[:], in_=strict_t[:])
    slow_sb = sb.tile([128, 128], F32)
    nc.sync.dma_start(out=slow_sb[:], in_=slow_t[:])
    ident_sb = sb.tile([128, 128], F32)
    nc.sync.dma_start(out=ident_sb[:], in_=ident_t[:])

    imi = sb.tile([128, 3], F32)
    nc.sync.dma_start(out=imi[:], in_=bass.AP(iminfo_t, 0, [[0, 128], [1, 3]]))
    imw1 = sb.tile([128, 1], F32)
    nc.vector.tensor_scalar(imw1[:], imi[:, 1:2], 1.0, None, OP.subtract)
    imh1 = sb.tile([128, 1], F32)
    nc.vector.tensor_scalar(imh1[:], imi[:, 0:1], 1.0, None, OP.subtract)
    minsz = sb.tile([128, 1], F32)
    nc.vector.tensor_scalar(minsz[:], imi[:, 2:3], 16.0, None, OP.mult)

    ones512 = sb.tile([128, SEL], F32)
    nc.vector.memset(ones512[:], 1.0)
    b17 = sb.tile([128, 1], F32)
    nc.vector.memset(b17[:], 1.7)
    b10 = sb.tile([128, 1], F32)
    nc.vector.memset(b10[:], 1.0)
    ones_col = sb.tile([128, 1], F32)
    nc.vector.memset(ones_col[:], 1.0)
    ones_row = sb.tile([1, 128], F32)
    nc.vector.memset(ones_row[:], 1.0)
    zeros14 = sb.tile([1, 4], F32)
    nc.vector.memset(zeros14[:], 0.0)

    # ---------------- phase 1: scores -> B [128,288] ----------------
    B = sb.tile([128, 288], F32)
    nc.vector.memset(B[:], NEG)
    nc.sync.dma_start(out=B[0:125, :],
                      in_=bass.AP(scores_t, NCAND, [[288, 125], [1, 288]]))

    kidx = sb.tile([128, 32], F32)
    nc.sync.dma_start(out=kidx[:], in_=kidx_t[:])
    pbase = sb.tile([128, 1], F32)
    nc.sync.dma_start(out=pbase[:], in_=pbase_t[:])
    zeros32 = sb.tile([128, 32], F32)
    nc.vector.memset(zeros32[:], 0.0)

    # ---------------- phase 2: per-partition top-32 + kth -> tau ----------------
    E = sb.tile([128, 32], F32)
    X = sb.tile([128, 32], U32)
    Bw = sb.tile([128, 288], F32)
    nc.vector.max(out=E[:, 0:8], in_=B[:])
    nc.vector.max_index(out=X[:, 0:8], in_max=E[:, 0:8], in_values=B[:])
    nc.vector.match_replace(out=Bw[:], in_to_replace=E[:, 0:8], in_values=B[:],
                            imm_value=ZAP)
    for r in (1, 2, 3):
        nc.vector.max(out=E[:, 8 * r:8 * r + 8], in_=Bw[:])
        nc.vector.max_index(out=X[:, 8 * r:8 * r + 8],
                            in_max=E[:, 8 * r:8 * r + 8], in_values=Bw[:])
        if r < 3:
            nc.vector.match_replace(out=Bw[:], in_to_replace=E[:, 8 * r:8 * r + 8],
                                    in_values=Bw[:], imm_value=ZAP)

    kv = sb.tile([1, 2], F32)
    nc.gpsimd.kth_largest(kv[:], E[:], n_per_lane=32, k=KADJ, quantile=_KTH_Q)
    psmall = ps_small.tile([128, 24], F32)
    nc.tensor.matmul(out=psmall[:, 0:1], lhsT=ones_row[:], rhs=kv[0:1, 1:2],
                     start=True, stop=True)
    tau128 = sb.tile([128, 1], F32)
    nc.vector.tensor_copy(tau128[:], psmall[:, 0:1])

    # ---------------- phase 3: prefix-sum compaction + indirect scatter ----------
    sel = sb.tile([128, 32], F32)
    nc.vector.tensor_scalar(sel[:], E[:], tau128[:], None, OP.is_gt)
    Xf = sb.tile([128, 32], F32)
    nc.vector.tensor_copy(Xf[:], X[:])
    cid_all = sb.tile([128, 32], F32)
    nc.vector.tensor_scalar(cid_all[:], Xf[:], pbase[:], None, OP.add)

    scn = sb.tile([128, 32], F32)
    nc.vector.tensor_tensor_scan(scn[:], data0=sel[:], data1=zeros32[:],
                                 initial=0.0, op0=OP.add, op1=OP.add)
    nc.tensor.matmul(out=psmall[:, 16:17], lhsT=slow_sb[:], rhs=scn[:, 31:32],
                     start=True, stop=True)
    nc.tensor.matmul(out=psmall[0:1, 17:18], lhsT=ones_col[:], rhs=scn[:, 31:32],
                     start=True, stop=True)
    posf = sb.tile([128, 32], F32)
    nc.vector.scalar_tensor_tensor(posf[:], in0=scn[:], scalar=psmall[:, 16:17],
                                   in1=sel[:], op0=OP.add, op1=OP.subtract)
    # dest = sel ? pos : 512 + (k - pos)   (all in-range; trash region collides ok)
    tt_ = sb.tile([128, 32], F32)
    nc.vector.tensor_tensor(tt_[:], kidx[:], posf[:], op=OP.subtract)
    nc.vector.tensor_scalar(tt_[:], tt_[:], float(SEL), None, OP.add)
    du = sb.tile([128, 32], F32)
    nc.vector.tensor_tensor(du[:], posf[:], tt_[:], op=OP.subtract)
    nc.vector.tensor_tensor(du[:], du[:], sel[:], op=OP.mult)
    nc.vector.tensor_tensor(du[:], du[:], tt_[:], op=OP.add)
    dsti = sb.tile([128, 32], I32)
    nc.vector.tensor_copy(dsti[:], du[:])

    pre = sb.tile([1, SEL], F32)
    nc.vector.memset(pre[:], -1.0)
    nc.sync.dma_start(out=bass.AP(s1_t, 0, [[1, SEL], [1, 1]]), in_=pre[:])
    nc.gpsimd.indirect_dma_start(
        out=bass.AP(s1_t, 0, [[1, SEL + 4096], [1, 1]]),
        out_offset=bass.IndirectOffsetOnAxis(ap=dsti[:], axis=0),
        in_=cid_all[:], in_offset=None)

    cids = sb.tile([128, 4], F32)
    nc.sync.dma_start(out=cids[:], in_=bass.AP(s1_t, 0, [[4, 128], [1, 4]]))

    # selection-health checks: intra-partition duplicate selected scores
    # (max_index ambiguity) and possible >32-per-partition truncation
    eqa = sb.tile([128, 32], F32)
    nc.vector.tensor_tensor(eqa[:, 0:31], E[:, 0:31], E[:, 1:32], op=OP.is_equal)
    nc.vector.tensor_tensor(eqa[:, 0:31], eqa[:, 0:31], sel[:, 1:32], op=OP.mult)
    nc.vector.tensor_scalar(eqa[:, 31:32], E[:, 31:32], tau128[:], None, OP.is_gt)
    rowbad = sb.tile([128, 1], F32)
    nc.vector.tensor_reduce(rowbad[:], eqa[:], axis=mybir.AxisListType.X,
                            op=OP.add)
    nc.tensor.matmul(out=psmall[0:1, 18:19], lhsT=ones_col[:], rhs=rowbad[:],
                     start=True, stop=True)

    padgood = sb.tile([128, 4], F32)
    nc.vector.tensor_scalar(padgood[:], cids[:], 0.0, None, OP.is_ge)
    padg_u = sb.tile([128, 4], U32)
    nc.vector.tensor_scalar(padg_u[:], cids[:], 0.0, None, OP.is_ge)
    cpos = sb.tile([128, 4], F32)
    nc.vector.tensor_scalar(cpos[:], cids[:], 0.0, None, OP.max)
    cidx = sb.tile([128, 4], I32)
    nc.vector.tensor_copy(cidx[:], cpos[:])

    # ---------------- phase 4: gathers ----------------
    rec = sb.tile([128, 4, 8], F32)
    nc.gpsimd.indirect_dma_start(
        out=rec[:], out_offset=None,
        in_=bass.AP(rec_t, 0, [[8, NPADC], [1, 8]]),
        in_offset=bass.IndirectOffsetOnAxis(ap=cidx[:], axis=0))

    soff = sb.tile([128, 4], I32)
    nc.vector.tensor_scalar(soff[:], cidx[:], NCAND, None, OP.add)
    scg = sb.tile([128, 4], F32)
    nc.gpsimd.indirect_dma_start(
        out=scg[:], out_offset=None,
        in_=bass.AP(scores_t, 0, [[1, 2 * A * NLOC], [1, 1]]),
        in_offset=bass.IndirectOffsetOnAxis(ap=soff[:], axis=0))

    d0i = sb.tile([128, 4], I32)
    nc.vector.tensor_copy(d0i[:], rec[:, :, 0])
    dgs = []
    for k in range(4):
        if k == 0:
            offk = d0i
        else:
            offk = sb.tile([128, 4], I32, tag=f"offk{k}")
            nc.vector.tensor_scalar(offk[:], d0i[:], k * NLOC, None, OP.add)
        dgk = sb.tile([128, 4], F32, tag=f"dg{k}")
        nc.gpsimd.indirect_dma_start(
            out=dgk[:], out_offset=None,
            in_=bass.AP(deltas_t, 0, [[1, 4 * A * NLOC], [1, 1]]),
            in_offset=bass.IndirectOffsetOnAxis(ap=offk[:], axis=0))
        dgs.append(dgk)

    # ---------------- phase 5: decode 512 boxes ----------------
    r_aw = rec[:, :, 2]
    r_ah = rec[:, :, 3]
    r_awh = rec[:, :, 4]
    r_ahh = rec[:, :, 5]
    r_acx = rec[:, :, 6]
    r_acy = rec[:, :, 7]

    ed = sb.tile([128, 8], F32)
    nc.vector.tensor_scalar(ed[:, 0:4], dgs[2][:], -10.0, 10.0, OP.max, OP.min)
    nc.vector.tensor_scalar(ed[:, 4:8], dgs[3][:], -10.0, 10.0, OP.max, OP.min)
    ex = sb.tile([128, 8], F32)
    nc.scalar.activation(ex[:], ed[:], AF.Exp)

    pwh = sb.tile([128, 4], F32)
    nc.vector.tensor_tensor(pwh[:], ex[:, 0:4], r_awh, op=OP.mult)
    phh = sb.tile([128, 4], F32)
    nc.vector.tensor_tensor(phh[:], ex[:, 4:8], r_ahh, op=OP.mult)

    pcx = sb.tile([128, 4], F32)
    nc.vector.tensor_tensor(pcx[:], dgs[0][:], r_aw, op=OP.mult)
    nc.vector.tensor_tensor(pcx[:], pcx[:], r_acx, op=OP.add)
    pcy = sb.tile([128, 4], F32)
    nc.vector.tensor_tensor(pcy[:], dgs[1][:], r_ah, op=OP.mult)
    nc.vector.tensor_tensor(pcy[:], pcy[:], r_acy, op=OP.add)

    x1 = sb.tile([128, 4], F32)
    nc.vector.tensor_tensor(x1[:], pcx[:], pwh[:], op=OP.subtract)
    x2 = sb.tile([128, 4], F32)
    nc.vector.tensor_tensor(x2[:], pcx[:], pwh[:], op=OP.add)
    y1 = sb.tile([128, 4], F32)
    nc.vector.tensor_tensor(y1[:], pcy[:], phh[:], op=OP.subtract)
    y2 = sb.tile([128, 4], F32)
    nc.vector.tensor_tensor(y2[:], pcy[:], phh[:], op=OP.add)

    nc.vector.tensor_scalar(x1[:], x1[:], 0.0, imw1[:], OP.max, OP.min)
    nc.vector.tensor_scalar(x2[:], x2[:], 0.0, imw1[:], OP.max, OP.min)
    nc.vector.tensor_scalar(y1[:], y1[:], 0.0, imh1[:], OP.max, OP.min)
    nc.vector.tensor_scalar(y2[:], y2[:], 0.0, imh1[:], OP.max, OP.min)

    wt = sb.tile([128, 4], F32)
    nc.vector.tensor_tensor(wt[:], x2[:], x1[:], op=OP.subtract)
    ht = sb.tile([128, 4], F32)
    nc.vector.tensor_tensor(ht[:], y2[:], y1[:], op=OP.subtract)
    vx = sb.tile([128, 4], F32)
    nc.vector.tensor_scalar(vx[:], wt[:], 1.0, minsz[:], OP.add, OP.is_ge)
    vy = sb.tile([128, 4], F32)
    nc.vector.tensor_scalar(vy[:], ht[:], 1.0, minsz[:], OP.add, OP.is_ge)
    alive0 = sb.tile([128, 4], F32)
    nc.vector.tensor_tensor(alive0[:], vx[:], vy[:], op=OP.mult)
    nc.vector.tensor_tensor(alive0[:], alive0[:], padgood[:], op=OP.mult)

    wp = sb.tile([128, 4], F32)
    nc.vector.tensor_scalar(wp[:], wt[:], 1.0, None, OP.add)
    hp = sb.tile([128, 4], F32)
    nc.vector.tensor_scalar(hp[:], ht[:], 1.0, None, OP.add)
    a7 = sb.tile([128, 4], F32)
    nc.vector.tensor_tensor(a7[:], wp[:], hp[:], op=OP.mult)
    nc.vector.tensor_scalar(a7[:], a7[:], float(NMS_THRESH), None, OP.mult)

    scm = sb.tile([128, 4], F32)
    nc.vector.memset(scm[:], NEG)
    nc.vector.copy_predicated(scm[:], padg_u[:], scg[:])

    # tie-break ids: gathered ref id where real, 40000+slot for pads (distinct)
    ids2 = sb.tile([128, 4], F32)
    nc.sync.dma_start(out=ids2[:], in_=slotid_t[:])
    nc.vector.copy_predicated(ids2[:], padg_u[:], rec[:, :, 1])

    # ---------------- phase 6: exact stable rank ----------------
    psc = ps_rows.tile([128, SEL], F32, tag="rows")
    pid = ps_rows.tile([128, SEL], F32, tag="rows")
    for c in range(4):
        nc.tensor.transpose(out=psc[:, 128 * c:128 * (c + 1)],
                            in_=scm[:, c:c + 1].to_broadcast([128, 128]),
                            identity=ident_sb[:])
        nc.tensor.transpose(out=pid[:, 128 * c:128 * (c + 1)],
                            in_=ids2[:, c:c + 1].to_broadcast([128, 128]),
                            identity=ident_sb[:])

    junk1 = sb.tile([128, SEL], F32)
    eqt = sb.tile([128, SEL], F32)
    rkgt = sb.tile([128, 4], F32)
    rktie = sb.tile([128, 4], F32)
    for c in range(4):
        nc.vector.scalar_tensor_tensor(junk1[:], in0=psc[:], scalar=scm[:, c:c + 1],
                                       in1=ones512[:], op0=OP.is_gt, op1=OP.mult,
                                       accum_out=rkgt[:, c:c + 1])
        nc.vector.tensor_scalar(eqt[:], psc[:], scm[:, c:c + 1], None, OP.is_equal)
        nc.vector.scalar_tensor_tensor(junk1[:], in0=pid[:], scalar=ids2[:, c:c + 1],
                                       in1=eqt[:], op0=OP.is_lt, op1=OP.mult,
                                       accum_out=rktie[:, c:c + 1])
    rank = sb.tile([128, 4], F32)
    nc.vector.tensor_tensor(rank[:], rkgt[:], rktie[:], op=OP.add)
    ranki = sb.tile([128, 4], I32)
    nc.vector.tensor_copy(ranki[:], rank[:])

    # ---------------- phase 7: reorder records by rank via DRAM ----------------
    reco = sb.tile([128, 4, 8], F32)
    nc.vector.memset(reco[:], 0.0)
    nc.vector.tensor_copy(reco[:, :, 0], x1[:])
    nc.vector.tensor_copy(reco[:, :, 1], y1[:])
    nc.vector.tensor_copy(reco[:, :, 2], x2[:])
    nc.vector.tensor_copy(reco[:, :, 3], y2[:])
    nc.vector.tensor_copy(reco[:, :, 4], a7[:])
    nc.vector.tensor_copy(reco[:, :, 5], alive0[:])
    nc.gpsimd.indirect_dma_start(
        out=bass.AP(s2_t, 0, [[8, SEL], [1, 8]]),
        out_offset=bass.IndirectOffsetOnAxis(ap=ranki[:], axis=0),
        in_=reco[:], in_offset=None)

    # reload in rank order r = c*128 + p
    ordb = sb.tile([128, 4, 8], F32)
    nc.sync.dma_start(out=ordb[:],
                      in_=bass.AP(s2_t, 0, [[8, 128], [1024, 4], [1, 8]]))

    RX1 = ps_rows.tile([128, SEL], F32, tag="rows")
    RY1 = ps_rows.tile([128, SEL], F32, tag="rows")
    RX2 = ps_rows.tile([128, SEL], F32, tag="rows")
    RY2 = ps_rows.tile([128, SEL], F32, tag="rows")
    RA7 = ps_rows.tile([128, SEL], F32, tag="rows")
    for (rt, fidx) in ((RX1, 0), (RY1, 1), (RX2, 2), (RY2, 3), (RA7, 4)):
        for c in range(4):
            nc.tensor.transpose(out=rt[:, 128 * c:128 * (c + 1)],
                                in_=ordb[:, c, fidx:fidx + 1].to_broadcast([128, 128]),
                                identity=ident_sb[:])

    # ---------------- phase 8: suppression masks ----------------
    M = []
    for c in range(4):
        f0 = 128 * c
        Wc = SEL - f0
        t1 = sbw.tile([128, SEL], F32, tag="mw")
        nc.vector.tensor_scalar(t1[:, 0:Wc], RX2[:, f0:SEL], ordb[:, c, 2:3],
                                None, OP.min)
        u = sbw.tile([128, SEL], F32, tag="mw")
        nc.vector.scalar_tensor_tensor(u[:, 0:Wc], in0=RX1[:, f0:SEL],
                                       scalar=ordb[:, c, 0:1], in1=t1[:, 0:Wc],
                                       op0=OP.max, op1=OP.subtract)
        iw = sbw.tile([128, SEL], F32, tag="mw2")
        nc.scalar.activation(iw[:, 0:Wc], u[:, 0:Wc], AF.Relu, bias=b17[:], scale=-1.7)
        t2 = sbw.tile([128, SEL], F32, tag="mw")
        nc.vector.tensor_scalar(t2[:, 0:Wc], RY2[:, f0:SEL], ordb[:, c, 3:4],
                                None, OP.min)
        vv = sbw.tile([128, SEL], F32, tag="mw")
        nc.vector.scalar_tensor_tensor(vv[:, 0:Wc], in0=RY1[:, f0:SEL],
                                       scalar=ordb[:, c, 1:2], in1=t2[:, 0:Wc],
                                       op0=OP.max, op1=OP.subtract)
        ih = sbw.tile([128, SEL], F32, tag="mw2")
        nc.scalar.activation(ih[:, 0:Wc], vv[:, 0:Wc], AF.Relu, bias=b10[:], scale=-1.0)
        it = sbw.tile([128, SEL], F32, tag="mw")
        nc.vector.tensor_tensor(it[:, 0:Wc], iw[:, 0:Wc], ih[:, 0:Wc], op=OP.mult)
        cm = sbw.tile([128, SEL], F32, tag="mw2")
        nc.vector.scalar_tensor_tensor(cm[:, 0:Wc], in0=RA7[:, f0:SEL],
                                       scalar=ordb[:, c, 4:5], in1=it[:, 0:Wc],
                                       op0=OP.add, op1=OP.is_lt)
        Mc = sb.tile([128, SEL], F32, tag=f"M{c}")
        nc.vector.tensor_tensor(Mc[:, 0:Wc], cm[:, 0:Wc],
                                strict_sb[:, c * SEL + f0:(c + 1) * SEL], op=OP.mult)
        M.append(Mc)

    # ---------------- phase 9: greedy NMS as fixpoint ----------------
    alv = ordb[:, :, 5]
    Ka = sb.tile([128, 4], F32, tag="Ka")
    Kb = sb.tile([128, 4], F32, tag="Kb")
    nc.vector.tensor_copy(Ka[:], alv)
    cur, nxt = Ka, Kb
    kept_hist = []
    for _t in range(ITERS + 1):
        for cj in range(4):
            for ci in range(cj + 1):
                nc.tensor.matmul(
                    out=psmall[:, cj:cj + 1],
                    lhsT=M[ci][:, (cj - ci) * 128:(cj - ci) * 128 + 128],
                    rhs=cur[:, ci:ci + 1],
                    start=(ci == 0), stop=(ci == cj))
        nc.vector.scalar_tensor_tensor(nxt[:], in0=psmall[:, 0:4], scalar=0.0,
                                       in1=alv, op0=OP.is_equal, op1=OP.mult)
        kept_hist.append((cur, nxt))
        cur, nxt = nxt, cur
    klast, kprev = kept_hist[-1][1], kept_hist[-1][0]

    dneq = sb.tile([128, 4], F32)
    nc.vector.tensor_tensor(dneq[:], klast[:], kprev[:], op=OP.not_equal)
    nc.tensor.matmul(out=psmall[0:1, 4:8], lhsT=ones_col[:], rhs=dneq[:],
                     start=True, stop=True)

    # ---------------- phase 10: output positions + scatter ----------------
    nc.tensor.matmul(out=psmall[:, 8:12], lhsT=slow_sb[:], rhs=klast[:],
                     start=True, stop=True)
    nc.tensor.matmul(out=psmall[0:1, 12:16], lhsT=ones_col[:], rhs=klast[:],
                     start=True, stop=True)
    cssb = sb.tile([1, 4], F32)
    nc.vector.tensor_copy(cssb[:], psmall[0:1, 12:16])
    incs = sb.tile([1, 4], F32)
    nc.vector.tensor_tensor_scan(incs[:], data0=cssb[:], data1=zeros14[:],
                                 initial=0.0, op0=OP.add, op1=OP.add)
    excs = sb.tile([1, 4], F32)
    nc.vector.tensor_tensor(excs[:], incs[:], cssb[:], op=OP.subtract)
    nc.tensor.matmul(out=psmall[:, 12:16], lhsT=ones_row[:], rhs=excs[:],
                     start=True, stop=True)
    cob = sb.tile([128, 4], F32)
    nc.vector.tensor_copy(cob[:], psmall[:, 12:16])

    dest = sb.tile([128, 4], F32)
    nc.vector.tensor_tensor(dest[:], psmall[:, 8:12], cob[:], op=OP.add)
    mk = sb.tile([128, 4], F32)
    nc.vector.scalar_tensor_tensor(mk[:], in0=dest[:], scalar=float(POST_NMS),
                                   in1=klast[:], op0=OP.is_lt, op1=OP.mult)
    nc.vector.scalar_tensor_tensor(dest[:], in0=dest[:], scalar=float(POST_NMS),
                                   in1=mk[:], op0=OP.subtract, op1=OP.mult)
    nc.vector.tensor_scalar(dest[:], dest[:], float(POST_NMS), None, OP.add)
    dri = sb.tile([128, 4], I32)
    nc.vector.tensor_copy(dri[:], dest[:])

    zer = sb.tile([1, 5 * (POST_NMS + 1)], F32)
    nc.vector.memset(zer[:], 0.0)
    nc.sync.dma_start(out=bass.AP(out_t, 0, [[5, POST_NMS + 1], [1, 5]]),
                      in_=zer[:])
    nc.gpsimd.indirect_dma_start(
        out=bass.AP(out_t, 0, [[5, POST_NMS + 1], [1, 5]]),
        out_offset=bass.IndirectOffsetOnAxis(ap=dri[:], axis=0),
        in_=ordb[:, :, 0:4], in_offset=None,
        element_offset=1)

    # ---------------- flags ----------------
    fl = sb.tile([1, 8], F32)
    nc.vector.memset(fl[:], 0.0)
    nc.vector.tensor_copy(fl[:, 0:4], psmall[0:1, 4:8])   # not-converged sums
    nc.vector.tensor_copy(fl[:, 4:5], incs[:, 3:4])       # total kept
    nc.vector.tensor_copy(fl[:, 5:6], psmall[0:1, 17:18])  # num selected
    nc.vector.tensor_copy(fl[:, 6:7], kv[:, 1:2])         # tau
    nc.vector.tensor_copy(fl[:, 7:8], psmall[0:1, 18:19])  # dup/trunc badness
    nc.sync.dma_start(out=flags_t[:], in_=fl[:])

    ctx.close()


# ---------------- host fallback: exact numpy replica of the reference ----------
def _reference_numpy(scores, bbox_deltas, im_info):
    base = _gen_anchors()
    sc = scores[:, A:, :, :].transpose(0, 2, 3, 1).reshape(-1).astype(np.float32)
    sx = (np.arange(W) * FEAT_STRIDE).astype(np.float32)
    sy = (np.arange(H) * FEAT_STRIDE).astype(np.float32)
    gy, gx = np.meshgrid(sy, sx, indexing="ij")
    shifts = np.stack([gx, gy, gx, gy], axis=-1).reshape(-1, 1, 4)
    anchors = (base[None, :, :] + shifts).reshape(-1, 4).astype(np.float32)
    d = bbox_deltas.transpose(0, 2, 3, 1).reshape(-1, 4).astype(np.float32)
    d = np.concatenate([d[:, :2], np.clip(d[:, 2:], -10.0, 10.0)], axis=1)
    aw = anchors[:, 2] - anchors[:, 0] + 1.0
    ah = anchors[:, 3] - anchors[:, 1] + 1.0
    acx = anchors[:, 0] + 0.5 * aw
    acy = anchors[:, 1] + 0.5 * ah
    pcx = d[:, 0] * aw + acx
    pcy = d[:, 1] * ah + acy
    pw = np.exp(d[:, 2]) * aw
    ph = np.exp(d[:, 3]) * ah
    boxes = np.stack([pcx - 0.5 * pw, pcy - 0.5 * ph,
                      pcx + 0.5 * pw, pcy + 0.5 * ph], axis=1).astype(np.float32)
    im_h, im_w, im_s = im_info[0]
    boxes = np.stack([np.clip(boxes[:, 0], 0, im_w - 1),
                      np.clip(boxes[:, 1], 0, im_h - 1),
                      np.clip(boxes[:, 2], 0, im_w - 1),
                      np.clip(boxes[:, 3], 0, im_h - 1)], axis=1).astype(np.float32)
    ws = boxes[:, 2] - boxes[:, 0] + 1.0
    hs = boxes[:, 3] - boxes[:, 1] + 1.0
    valid = (ws >= MIN_SIZE * im_s) & (hs >= MIN_SIZE * im_s)
    scm = np.where(valid, sc, -np.inf).astype(np.float32)
    order = np.argsort(-scm, kind="stable")[:PRE_NMS]
    top_sc = scm[order]
    props = boxes[order]
    x1, y1, x2, y2 = props[:, 0], props[:, 1], props[:, 2], props[:, 3]
    areas = (x2 - x1 + 1.0) * (y2 - y1 + 1.0)
    keep = np.isfinite(top_sc)
    for i in range(PRE_NMS):
        if not keep[i]:
            continue
        xx1 = np.maximum(x1[i], x1[i + 1:])
        yy1 = np.maximum(y1[i], y1[i + 1:])
        xx2 = np.minimum(x2[i], x2[i + 1:])
        yy2 = np.minimum(y2[i], y2[i + 1:])
        iw = np.clip(xx2 - xx1 + 1.0, 0.0, None)
        ih = np.clip(yy2 - yy1 + 1.0, 0.0, None)
        inter = iw * ih
        iou = inter / (areas[i] + areas[i + 1:] - inter)
        keep[i + 1:] &= ~(iou > NMS_THRESH)
    out = np.zeros((POST_NMS, 5), dtype=np.float32)
    kk = np.where(keep)[0][:POST_NMS]
    out[np.arange(len(kk)), 1:] = props[kk]
    return out


_PROGRAM = None


def _get_program():
    global _PROGRAM
    if _PROGRAM is None:
        _PROGRAM = build_program()
    return _PROGRAM


def make_in_map(scores, bbox_deltas, im_info):
    return {
        "scores": np.ascontiguousarray(scores, dtype=np.float32),
        "bbox_deltas": np.ascontiguousarray(bbox_deltas, dtype=np.float32),
        "im_info": np.ascontiguousarray(im_info, dtype=np.float32),
        "recs": _REC,
        "strict": _STRICT,
        "slow": _SLOW,
        "ident": _IDENT,
        "slotid": _SLOTID,
        "kidx": _KIDX,
        "pbase": _PBASE,
    }


def kernel(scores, bbox_deltas, im_info):
    nc = _get_program()
    in_map = make_in_map(scores, bbox_deltas, im_info)
    res = run_bass_kernel_spmd(nc, [in_map] * 8, core_ids=list(range(8)))
    r0 = res.results[0]
    out = np.asarray(r0["out"])[:POST_NMS]
    flags = np.asarray(r0["flags"]).reshape(-1)
    notconv = flags[0:4].sum() > 0
    kept = flags[4]
    nsel = flags[5]
    bad = flags[7]
    if notconv or kept < POST_NMS or nsel > SEL or bad > 0:
        out = _reference_numpy(np.asarray(scores), np.asarray(bbox_deltas),
                               np.asarray(im_info))
    return out.astype(np.float32)


# revision 13
# speedup vs baseline: 4.4054x; 4.4054x over previous
"""Trainium2 Bass kernel for nn_ProposalLayer (Faster-RCNN proposal layer).

Pipeline (single NeuronCore, replicated SPMD across 8 cores):
  1. Load raw fg scores [9*4000] into SBUF layout B [128,288] (flat j = a*4000+loc).
  2. Per-partition top-32 extraction (max8 + match_replace), then gpsimd
     kth_largest on the [128,32] extract -> exact 512th-largest raw score tau.
  3. payload = j where score_j > tau else -1; gpsimd sparse_gather compacts the
     selected ~511 candidate indices into 512 slots.
  4. Indirect-DMA gathers: per-candidate record table (host const: delta
     offsets, ref id, anchor params), the 4 deltas, and the score.
  5. Decode + clip + min-size validity for the 512 selected boxes only.
  6. Exact stable rank by (score desc, ref-id asc) via PE-transpose row
     broadcast + compare/accumulate; reorder slot records by rank through a
     DRAM bounce (indirect scatter).
  7. IoU>0.7 suppression masks (vs rank order), greedy NMS solved as a
     fixpoint of masked mat-vec products on the PE (converges in ~4 iters;
     an extra iteration verifies convergence -> flag).
  8. Output positions for the first 300 kept via triangular-matmul prefix
     sums; indirect scatter of box coords into out[301,5] rows (col 0 = 0).

Every data-dependent assumption (selection count <= 512, fixpoint converged,
>=300 kept) is checked on device into a FLAGS output; if any flag trips the
host falls back to an exact numpy replica of the reference (never on the
nominal input distribution).
"""

import numpy as np

import concourse.bass as bass
import concourse.bacc as bacc
import concourse.mybir as mybir
import concourse.tile as tile
from concourse.tile import add_dep_helper
from concourse.bass_utils import run_bass_kernel_spmd

F32 = mybir.dt.float32
I32 = mybir.dt.int32
U32 = mybir.dt.uint32
OP = mybir.AluOpType
AF = mybir.ActivationFunctionType

# ---------------- problem constants (hardcoded) ----------------
A = 9
H, W = 50, 80
NLOC = H * W          # 4000
NCAND = A * NLOC      # 36000
NPADC = 128 * 288     # 36864 padded candidates
FEAT_STRIDE = 16
PRE_NMS = 6000
POST_NMS = 300
NMS_THRESH = 0.7
MIN_SIZE = 16.0
NEG = -1.0e20         # "minus inf" stand-in (> -1e29 so kth counts it)
ZAP = -1.0e30         # match_replace fill

SEL = 512             # selected-candidate slots
SCAT = 16             # extraction columns fed to the compaction scatter
ITERS = 6             # NMS fixpoint iterations (+1 verification round)


def _gen_anchors():
    RATIOS = np.array([0.5, 1.0, 2.0])
    SCALES = np.array([8.0, 16.0, 32.0])

    def _whctrs(a):
        w = a[2] - a[0] + 1.0
        h = a[3] - a[1] + 1.0
        return w, h, a[0] + 0.5 * (w - 1.0), a[1] + 0.5 * (h - 1.0)

    def _mk(ws, hs, cx, cy):
        return np.stack([cx - 0.5 * (ws - 1.0), cy - 0.5 * (hs - 1.0),
                         cx + 0.5 * (ws - 1.0), cy + 0.5 * (hs - 1.0)], axis=1)

    base = np.array([1.0, 1.0, 16.0, 16.0]) - 1.0
    w, h, cx, cy = _whctrs(base)
    size = w * h
    ws = np.round(np.sqrt(size / RATIOS))
    hs = np.round(ws * RATIOS)
    ratio_anchors = _mk(ws, hs, cx, cy)
    out = []
    for ra in ratio_anchors:
        w, h, cx, cy = _whctrs(ra)
        out.append(_mk(w * SCALES, h * SCALES, cx, cy))
    return np.concatenate(out, axis=0).astype(np.float32)  # [9,4]


def _build_tables():
    """Host-constant tables, all keyed by flat j = a*4000 + loc."""
    base = _gen_anchors()  # [9,4] f32
    j = np.arange(NPADC)
    a = np.minimum(j // NLOC, A - 1)
    loc = j % NLOC
    hh = loc // W
    ww = loc % W
    gx = (ww * FEAT_STRIDE).astype(np.float32)
    gy = (hh * FEAT_STRIDE).astype(np.float32)
    # anchor box, f32 math mirrors the reference exactly (all values exact)
    x1 = base[a, 0] + gx
    y1 = base[a, 1] + gy
    x2 = base[a, 2] + gx
    y2 = base[a, 3] + gy
    aw = (x2 - x1 + np.float32(1.0)).astype(np.float32)
    ah = (y2 - y1 + np.float32(1.0)).astype(np.float32)
    acx = (x1 + np.float32(0.5) * aw).astype(np.float32)
    acy = (y1 + np.float32(0.5) * ah).astype(np.float32)
    offd = (a * 4 * NLOC + loc).astype(np.float32)       # dx offset in deltas flat
    idref = (loc * A + a).astype(np.float32)             # reference candidate id
    rec = np.stack([offd, idref, aw, ah,
                    (aw * np.float32(0.5)).astype(np.float32),
                    (ah * np.float32(0.5)).astype(np.float32),
                    acx, acy], axis=1).astype(np.float32)  # [NPADC, 8]
    rec[NCAND:] = 0.0

    # strict-order tiles: for i-group c (i = c*128 + p), cols f in [0,512):
    # strict[c][p, f] = 1.0 iff f > c*128 + p.  Stored as [128, 4*512].
    strict = np.zeros((128, 4 * SEL), dtype=np.float32)
    p = np.arange(128)[:, None]
    f = np.arange(SEL)[None, :]
    for c in range(4):
        strict[:, c * SEL:(c + 1) * SEL] = (f > (c * 128 + p)).astype(np.float32)

    # strictly-lower ones (lhsT for exclusive prefix): low[k, m] = 1 iff k < m
    k = np.arange(128)[:, None]
    m = np.arange(128)[None, :]
    slow = (k < m).astype(np.float32)
    ident = np.eye(128, dtype=np.float32)
    # distinct pad ids for the 512 slots: 40000 + (p*4 + c)
    slotid = (40000.0 + np.arange(SEL, dtype=np.float64).reshape(128, 4)).astype(np.float32)
    # flat entry index k = p*32 + c over the extraction tile
    kidx = np.arange(128 * 32, dtype=np.float64).reshape(128, 32).astype(np.float32)
    # per-partition candidate base j = p*288
    pbase = (np.arange(128, dtype=np.float64) * 288.0).reshape(128, 1).astype(np.float32)
    return rec, strict, slow, ident, slotid, kidx, pbase


_REC, _STRICT, _SLOW, _IDENT, _SLOTID, _KIDX, _PBASE = _build_tables()


def build_program():
    nc = bacc.Bacc("TRN2", debug=False, enable_asserts=False, num_devices=8)

    scores_t = nc.dram_tensor("scores", [1, 2 * A, H, W], F32, kind="ExternalInput")
    deltas_t = nc.dram_tensor("bbox_deltas", [1, 4 * A, H, W], F32, kind="ExternalInput")
    iminfo_t = nc.dram_tensor("im_info", [1, 3], F32, kind="ExternalInput")
    rec_t = nc.dram_tensor("recs", [NPADC, 8], F32, kind="ExternalInput")
    strict_t = nc.dram_tensor("strict", [128, 4 * SEL], F32, kind="ExternalInput")
    slow_t = nc.dram_tensor("slow", [128, 128], F32, kind="ExternalInput")
    ident_t = nc.dram_tensor("ident", [128, 128], F32, kind="ExternalInput")
    slotid_t = nc.dram_tensor("slotid", [128, 4], F32, kind="ExternalInput")
    kidx_t = nc.dram_tensor("kidx", [128, 32], F32, kind="ExternalInput")
    pbase_t = nc.dram_tensor("pbase", [128, 1], F32, kind="ExternalInput")

    s1_t = nc.dram_tensor("s1", [SEL + 4096, 1], F32)          # cid scatter (+trash)
    s2_t = nc.dram_tensor("s2", [SEL, 8], F32)                 # reorder bounce
    out_t = nc.dram_tensor("out", [POST_NMS + 1, 5], F32, kind="ExternalOutput")
    flags_t = nc.dram_tensor("flags", [1, 8], F32, kind="ExternalOutput")

    with tile.TileContext(nc) as tc:
        _body(tc, scores_t, deltas_t, iminfo_t, rec_t, strict_t, slow_t,
              ident_t, slotid_t, kidx_t, pbase_t, s1_t, s2_t, out_t, flags_t)
    nc.compile()
    return nc


def _body(tc, scores_t, deltas_t, iminfo_t, rec_t, strict_t, slow_t, ident_t,
          slotid_t, kidx_t, pbase_t, s1_t, s2_t, out_t, flags_t):
    nc = tc.nc
    import contextlib
    ctx = contextlib.ExitStack()
    sb = ctx.enter_context(tc.tile_pool(name="sb", bufs=1))
    sbw = ctx.enter_context(tc.tile_pool(name="sbw", bufs=3))   # mask scratch
    ps_rows = ctx.enter_context(tc.tile_pool(name="psrows", bufs=7, space="PSUM"))
    ps_small = ctx.enter_context(tc.tile_pool(name="pssmall", bufs=1, space="PSUM"))

    # ---------------- constants in ----------------
    strict_sb = sb.tile([128, 4 * SEL], F32)
    nc.sync.dma_start(out=strict_sb[:], in_=strict_t[:])
    slow_sb = sb.tile([128, 128], F32)
    nc.sync.dma_start(out=slow_sb[:], in_=slow_t[:])
    ident_sb = sb.tile([128, 128], F32)
    nc.sync.dma_start(out=ident_sb[:], in_=ident_t[:])

    imi = sb.tile([128, 3], F32)
    nc.sync.dma_start(out=imi[:], in_=bass.AP(iminfo_t, 0, [[0, 128], [1, 3]]))
    imw1 = sb.tile([128, 1], F32)
    nc.vector.tensor_scalar(imw1[:], imi[:, 1:2], 1.0, None, OP.subtract)
    imh1 = sb.tile([128, 1], F32)
    nc.vector.tensor_scalar(imh1[:], imi[:, 0:1], 1.0, None, OP.subtract)
    minsz = sb.tile([128, 1], F32)
    nc.vector.tensor_scalar(minsz[:], imi[:, 2:3], 16.0, None, OP.mult)

    ones512 = sb.tile([128, SEL], F32)
    nc.vector.memset(ones512[:], 1.0)
    b17 = sb.tile([128, 1], F32)
    nc.vector.memset(b17[:], 1.7)
    b10 = sb.tile([128, 1], F32)
    nc.vector.memset(b10[:], 1.0)
    ones_col = sb.tile([128, 1], F32)
    nc.vector.memset(ones_col[:], 1.0)
    ones_row = sb.tile([1, 128], F32)
    nc.vector.memset(ones_row[:], 1.0)
    zeros14 = sb.tile([1, 4], F32)
    nc.vector.memset(zeros14[:], 0.0)

    # ---------------- phase 1: scores -> B [128,288] ----------------
    B = sb.tile([128, 288], F32)
    nc.vector.memset(B[:], NEG)
    nc.sync.dma_start(out=B[0:125, :],
                      in_=bass.AP(scores_t, NCAND, [[288, 125], [1, 288]]))

    kidx = sb.tile([128, 32], F32)
    nc.sync.dma_start(out=kidx[:], in_=kidx_t[:])
    pbase = sb.tile([128, 1], F32)
    nc.sync.dma_start(out=pbase[:], in_=pbase_t[:])
    zeros32 = sb.tile([128, 32], F32)
    nc.vector.memset(zeros32[:], 0.0)

    # ---------------- phase 2: per-partition top-32 + kth -> tau ----------------
    E = sb.tile([128, 32], F32)
    X = sb.tile([128, 32], U32)
    Bw = sb.tile([128, 288], F32)
    nc.vector.max(out=E[:, 0:8], in_=B[:])
    nc.vector.max_index(out=X[:, 0:8], in_max=E[:, 0:8], in_values=B[:])
    nc.vector.match_replace(out=Bw[:], in_to_replace=E[:, 0:8], in_values=B[:],
                            imm_value=ZAP)
    for r in (1, 2, 3):
        nc.vector.max(out=E[:, 8 * r:8 * r + 8], in_=Bw[:])
        nc.vector.max_index(out=X[:, 8 * r:8 * r + 8],
                            in_max=E[:, 8 * r:8 * r + 8], in_values=Bw[:])
        if r < 3:
            nc.vector.match_replace(out=Bw[:], in_to_replace=E[:, 8 * r:8 * r + 8],
                                    in_values=Bw[:], imm_value=ZAP)

    # tau: pick among 128 candidate thresholds t_p = 4th-largest of partition p
    # the one whose global exceedance count is the largest value <= SEL-1.
    psmall = ps_small.tile([128, 24], F32)
    TB = ps_rows.tile([128, 128], F32, tag="rows")
    nc.tensor.transpose(out=TB[:], in_=E[:, 3:4].to_broadcast([128, 128]),
                        identity=ident_sb[:])
    ACC = sb.tile([128, 128], F32)
    nc.vector.memset(ACC[:], 0.0)
    for c in range(32):
        nc.vector.scalar_tensor_tensor(ACC[:], in0=TB[:], scalar=E[:, c:c + 1],
                                       in1=ACC[:], op0=OP.is_lt, op1=OP.add)
    nc.tensor.matmul(out=psmall[:, 19:20], lhsT=ACC[:], rhs=ones_col[:],
                     start=True, stop=True)
    cnt = sb.tile([128, 1], F32)
    nc.vector.tensor_copy(cnt[:], psmall[:, 19:20])
    okc = sb.tile([128, 1], F32)
    nc.vector.tensor_scalar(okc[:], cnt[:], float(SEL - 1), None, OP.is_le)
    val = sb.tile([128, 1], F32)
    nc.vector.tensor_tensor(val[:], cnt[:], okc[:], op=OP.mult)
    nc.vector.tensor_tensor(val[:], val[:], okc[:], op=OP.add)   # sel: cnt+1, else 0
    VB = ps_rows.tile([128, 128], F32, tag="rows")
    nc.tensor.transpose(out=VB[:], in_=val[:].to_broadcast([128, 128]),
                        identity=ident_sb[:])
    vrow = sb.tile([1, 128], F32)
    nc.vector.tensor_copy(vrow[:], VB[0:1, :])
    mb8 = sb.tile([1, 8], F32)
    nc.vector.max(out=mb8[:], in_=vrow[:])
    # broadcast best (cnt+1) to all partitions
    nc.tensor.matmul(out=psmall[:, 20:21], lhsT=ones_row[:], rhs=mb8[0:1, 0:1],
                     start=True, stop=True)
    mbest = sb.tile([128, 1], F32)
    nc.vector.tensor_copy(mbest[:], psmall[:, 20:21])
    nc.vector.tensor_scalar(mbest[:], mbest[:], 1.0, None, OP.subtract)
    tmask = sb.tile([128, 1], F32)
    nc.vector.tensor_tensor(tmask[:], cnt[:], mbest[:], op=OP.is_equal)
    nc.vector.tensor_tensor(tmask[:], tmask[:], okc[:], op=OP.mult)
    tmu = sb.tile([128, 1], U32)
    nc.vector.tensor_scalar(tmu[:], tmask[:], 0.0, None, OP.is_gt)
    tcand = sb.tile([128, 1], F32)
    nc.vector.memset(tcand[:], -1e30)
    nc.vector.copy_predicated(tcand[:], tmu[:], E[:, 3:4])
    TC = ps_rows.tile([128, 128], F32, tag="rows")
    nc.tensor.transpose(out=TC[:], in_=tcand[:].to_broadcast([128, 128]),
                        identity=ident_sb[:])
    trow = sb.tile([1, 128], F32)
    nc.vector.tensor_copy(trow[:], TC[0:1, :])
    kv = sb.tile([1, 8], F32)
    nc.vector.max(out=kv[:], in_=trow[:])
    nc.tensor.matmul(out=psmall[:, 0:1], lhsT=ones_row[:], rhs=kv[0:1, 0:1],
                     start=True, stop=True)
    tau128 = sb.tile([128, 1], F32)
    nc.vector.tensor_copy(tau128[:], psmall[:, 0:1])

    # ---------------- phase 3: prefix-sum compaction + indirect scatter ----------
    sel = sb.tile([128, SCAT], F32)
    nc.vector.tensor_scalar(sel[:], E[:, 0:SCAT], tau128[:], None, OP.is_gt)
    Xf = sb.tile([128, SCAT], F32)
    nc.vector.tensor_copy(Xf[:], X[:, 0:SCAT])
    cid_all = sb.tile([128, SCAT], F32)
    nc.vector.tensor_scalar(cid_all[:], Xf[:], pbase[:], None, OP.add)

    scn = sb.tile([128, SCAT], F32)
    nc.vector.tensor_tensor_scan(scn[:], data0=sel[:], data1=zeros32[:, 0:SCAT],
                                 initial=0.0, op0=OP.add, op1=OP.add)
    nc.tensor.matmul(out=psmall[:, 16:17], lhsT=slow_sb[:], rhs=scn[:, SCAT - 1:SCAT],
                     start=True, stop=True)
    nc.tensor.matmul(out=psmall[0:1, 17:18], lhsT=ones_col[:], rhs=scn[:, SCAT - 1:SCAT],
                     start=True, stop=True)
    posf = sb.tile([128, SCAT], F32)
    nc.vector.scalar_tensor_tensor(posf[:], in0=scn[:], scalar=psmall[:, 16:17],
                                   in1=sel[:], op0=OP.add, op1=OP.subtract)
    # dest = sel ? pos : 512 + (k - pos)   (all in-range; trash region collides ok)
    tt_ = sb.tile([128, SCAT], F32)
    nc.vector.tensor_tensor(tt_[:], kidx[:, 0:SCAT], posf[:], op=OP.subtract)
    nc.vector.tensor_scalar(tt_[:], tt_[:], float(SEL), None, OP.add)
    du = sb.tile([128, SCAT], F32)
    nc.vector.tensor_tensor(du[:], posf[:], tt_[:], op=OP.subtract)
    nc.vector.tensor_tensor(du[:], du[:], sel[:], op=OP.mult)
    nc.vector.tensor_tensor(du[:], du[:], tt_[:], op=OP.add)
    dsti = sb.tile([128, SCAT], I32)
    nc.vector.tensor_copy(dsti[:], du[:])

    pre = sb.tile([1, SEL], F32)
    nc.vector.memset(pre[:], -1.0)
    i_pre = nc.sync.dma_start(out=bass.AP(s1_t, 0, [[1, SEL], [1, 1]]), in_=pre[:])
    i_sc1 = nc.gpsimd.indirect_dma_start(
        out=bass.AP(s1_t, 0, [[1, SEL + 4096], [1, 1]]),
        out_offset=bass.IndirectOffsetOnAxis(ap=dsti[:], axis=0),
        in_=cid_all[:], in_offset=None)
    add_dep_helper(i_sc1.ins, i_pre.ins, reason="scatter after s1 prefill")

    cids = sb.tile([128, 4], F32)
    i_rd1 = nc.sync.dma_start(out=cids[:], in_=bass.AP(s1_t, 0, [[4, 128], [1, 4]]))
    add_dep_helper(i_rd1.ins, i_sc1.ins, reason="cids reload after s1 scatter")

    # selection-health checks: intra-partition duplicate selected scores
    # (max_index ambiguity) and possible >SCAT-per-partition truncation
    eqa = sb.tile([128, SCAT], F32)
    nc.vector.tensor_tensor(eqa[:, 0:SCAT - 1], E[:, 0:SCAT - 1], E[:, 1:SCAT],
                            op=OP.is_equal)
    nc.vector.tensor_tensor(eqa[:, 0:SCAT - 1], eqa[:, 0:SCAT - 1],
                            sel[:, 1:SCAT], op=OP.mult)
    nc.vector.tensor_scalar(eqa[:, SCAT - 1:SCAT], E[:, SCAT - 1:SCAT],
                            tau128[:], None, OP.is_gt)
    rowbad = sb.tile([128, 1], F32)
    nc.vector.tensor_reduce(rowbad[:], eqa[:], axis=mybir.AxisListType.X,
                            op=OP.add)
    nc.tensor.matmul(out=psmall[0:1, 18:19], lhsT=ones_col[:], rhs=rowbad[:],
                     start=True, stop=True)

    padgood = sb.tile([128, 4], F32)
    nc.vector.tensor_scalar(padgood[:], cids[:], 0.0, None, OP.is_ge)
    padg_u = sb.tile([128, 4], U32)
    nc.vector.tensor_scalar(padg_u[:], cids[:], 0.0, None, OP.is_ge)
    cpos = sb.tile([128, 4], F32)
    nc.vector.tensor_scalar(cpos[:], cids[:], 0.0, None, OP.max)
    cidx = sb.tile([128, 4], I32)
    nc.vector.tensor_copy(cidx[:], cpos[:])

    # ---------------- phase 4: gathers ----------------
    rec = sb.tile([128, 4, 8], F32)
    nc.gpsimd.indirect_dma_start(
        out=rec[:], out_offset=None,
        in_=bass.AP(rec_t, 0, [[8, NPADC], [1, 8]]),
        in_offset=bass.IndirectOffsetOnAxis(ap=cidx[:], axis=0))

    soff = sb.tile([128, 4], I32)
    nc.vector.tensor_scalar(soff[:], cidx[:], NCAND, None, OP.add)
    scg = sb.tile([128, 4], F32)
    nc.gpsimd.indirect_dma_start(
        out=scg[:], out_offset=None,
        in_=bass.AP(scores_t, 0, [[1, 2 * A * NLOC], [1, 1]]),
        in_offset=bass.IndirectOffsetOnAxis(ap=soff[:], axis=0))

    d0i = sb.tile([128, 4], I32)
    nc.vector.tensor_copy(d0i[:], rec[:, :, 0])
    dgs = []
    for k in range(4):
        if k == 0:
            offk = d0i
        else:
            offk = sb.tile([128, 4], I32, tag=f"offk{k}")
            nc.vector.tensor_scalar(offk[:], d0i[:], k * NLOC, None, OP.add)
        dgk = sb.tile([128, 4], F32, tag=f"dg{k}")
        nc.gpsimd.indirect_dma_start(
            out=dgk[:], out_offset=None,
            in_=bass.AP(deltas_t, 0, [[1, 4 * A * NLOC], [1, 1]]),
            in_offset=bass.IndirectOffsetOnAxis(ap=offk[:], axis=0))
        dgs.append(dgk)

    # ---------------- phase 5: decode 512 boxes ----------------
    r_aw = rec[:, :, 2]
    r_ah = rec[:, :, 3]
    r_awh = rec[:, :, 4]
    r_ahh = rec[:, :, 5]
    r_acx = rec[:, :, 6]
    r_acy = rec[:, :, 7]

    ed = sb.tile([128, 8], F32)
    nc.vector.tensor_scalar(ed[:, 0:4], dgs[2][:], -10.0, 10.0, OP.max, OP.min)
    nc.vector.tensor_scalar(ed[:, 4:8], dgs[3][:], -10.0, 10.0, OP.max, OP.min)
    ex = sb.tile([128, 8], F32)
    nc.scalar.activation(ex[:], ed[:], AF.Exp)

    pwh = sb.tile([128, 4], F32)
    nc.vector.tensor_tensor(pwh[:], ex[:, 0:4], r_awh, op=OP.mult)
    phh = sb.tile([128, 4], F32)
    nc.vector.tensor_tensor(phh[:], ex[:, 4:8], r_ahh, op=OP.mult)

    pcx = sb.tile([128, 4], F32)
    nc.vector.tensor_tensor(pcx[:], dgs[0][:], r_aw, op=OP.mult)
    nc.vector.tensor_tensor(pcx[:], pcx[:], r_acx, op=OP.add)
    pcy = sb.tile([128, 4], F32)
    nc.vector.tensor_tensor(pcy[:], dgs[1][:], r_ah, op=OP.mult)
    nc.vector.tensor_tensor(pcy[:], pcy[:], r_acy, op=OP.add)

    x1 = sb.tile([128, 4], F32)
    nc.vector.tensor_tensor(x1[:], pcx[:], pwh[:], op=OP.subtract)
    x2 = sb.tile([128, 4], F32)
    nc.vector.tensor_tensor(x2[:], pcx[:], pwh[:], op=OP.add)
    y1 = sb.tile([128, 4], F32)
    nc.vector.tensor_tensor(y1[:], pcy[:], phh[:], op=OP.subtract)
    y2 = sb.tile([128, 4], F32)
    nc.vector.tensor_tensor(y2[:], pcy[:], phh[:], op=OP.add)

    nc.vector.tensor_scalar(x1[:], x1[:], 0.0, imw1[:], OP.max, OP.min)
    nc.vector.tensor_scalar(x2[:], x2[:], 0.0, imw1[:], OP.max, OP.min)
    nc.vector.tensor_scalar(y1[:], y1[:], 0.0, imh1[:], OP.max, OP.min)
    nc.vector.tensor_scalar(y2[:], y2[:], 0.0, imh1[:], OP.max, OP.min)

    wt = sb.tile([128, 4], F32)
    nc.vector.tensor_tensor(wt[:], x2[:], x1[:], op=OP.subtract)
    ht = sb.tile([128, 4], F32)
    nc.vector.tensor_tensor(ht[:], y2[:], y1[:], op=OP.subtract)
    vx = sb.tile([128, 4], F32)
    nc.vector.tensor_scalar(vx[:], wt[:], 1.0, minsz[:], OP.add, OP.is_ge)
    vy = sb.tile([128, 4], F32)
    nc.vector.tensor_scalar(vy[:], ht[:], 1.0, minsz[:], OP.add, OP.is_ge)
    alive0 = sb.tile([128, 4], F32)
    nc.vector.tensor_tensor(alive0[:], vx[:], vy[:], op=OP.mult)
    nc.vector.tensor_tensor(alive0[:], alive0[:], padgood[:], op=OP.mult)

    wp = sb.tile([128, 4], F32)
    nc.vector.tensor_scalar(wp[:], wt[:], 1.0, None, OP.add)
    hp = sb.tile([128, 4], F32)
    nc.vector.tensor_scalar(hp[:], ht[:], 1.0, None, OP.add)
    a7 = sb.tile([128, 4], F32)
    nc.vector.tensor_tensor(a7[:], wp[:], hp[:], op=OP.mult)
    nc.vector.tensor_scalar(a7[:], a7[:], float(NMS_THRESH), None, OP.mult)

    scm = sb.tile([128, 4], F32)
    nc.vector.memset(scm[:], NEG)
    nc.vector.copy_predicated(scm[:], padg_u[:], scg[:])

    # tie-break ids: gathered ref id where real, 40000+slot for pads (distinct)
    ids2 = sb.tile([128, 4], F32)
    nc.sync.dma_start(out=ids2[:], in_=slotid_t[:])
    nc.vector.copy_predicated(ids2[:], padg_u[:], rec[:, :, 1])

    # ---------------- phase 6: exact stable rank ----------------
    psc = ps_rows.tile([128, SEL], F32, tag="rows")
    pid = ps_rows.tile([128, SEL], F32, tag="rows")
    for c in range(4):
        nc.tensor.transpose(out=psc[:, 128 * c:128 * (c + 1)],
                            in_=scm[:, c:c + 1].to_broadcast([128, 128]),
                            identity=ident_sb[:])
        nc.tensor.transpose(out=pid[:, 128 * c:128 * (c + 1)],
                            in_=ids2[:, c:c + 1].to_broadcast([128, 128]),
                            identity=ident_sb[:])

    junk1 = sb.tile([128, SEL], F32)
    eqt = sb.tile([128, SEL], F32)
    rkgt = sb.tile([128, 4], F32)
    rktie = sb.tile([128, 4], F32)
    for c in range(4):
        nc.vector.scalar_tensor_tensor(junk1[:], in0=psc[:], scalar=scm[:, c:c + 1],
                                       in1=ones512[:], op0=OP.is_gt, op1=OP.mult,
                                       accum_out=rkgt[:, c:c + 1])
        nc.vector.tensor_scalar(eqt[:], psc[:], scm[:, c:c + 1], None, OP.is_equal)
        nc.vector.scalar_tensor_tensor(junk1[:], in0=pid[:], scalar=ids2[:, c:c + 1],
                                       in1=eqt[:], op0=OP.is_lt, op1=OP.mult,
                                       accum_out=rktie[:, c:c + 1])
    rank = sb.tile([128, 4], F32)
    nc.vector.tensor_tensor(rank[:], rkgt[:], rktie[:], op=OP.add)
    ranki = sb.tile([128, 4], I32)
    nc.vector.tensor_copy(ranki[:], rank[:])

    # ---------------- phase 7: reorder records by rank via DRAM ----------------
    reco = sb.tile([128, 4, 8], F32)
    nc.vector.memset(reco[:], 0.0)
    nc.vector.tensor_copy(reco[:, :, 0], x1[:])
    nc.vector.tensor_copy(reco[:, :, 1], y1[:])
    nc.vector.tensor_copy(reco[:, :, 2], x2[:])
    nc.vector.tensor_copy(reco[:, :, 3], y2[:])
    nc.vector.tensor_copy(reco[:, :, 4], a7[:])
    nc.vector.tensor_copy(reco[:, :, 5], alive0[:])
    i_sc2 = nc.gpsimd.indirect_dma_start(
        out=bass.AP(s2_t, 0, [[8, SEL], [1, 8]]),
        out_offset=bass.IndirectOffsetOnAxis(ap=ranki[:], axis=0),
        in_=reco[:], in_offset=None)

    # reload in rank order r = c*128 + p
    ordb = sb.tile([128, 4, 8], F32)
    i_rd2 = nc.sync.dma_start(out=ordb[:],
                              in_=bass.AP(s2_t, 0, [[8, 128], [1024, 4], [1, 8]]))
    add_dep_helper(i_rd2.ins, i_sc2.ins, reason="ordb reload after s2 scatter")

    RX1 = ps_rows.tile([128, SEL], F32, tag="rows")
    RY1 = ps_rows.tile([128, SEL], F32, tag="rows")
    RX2 = ps_rows.tile([128, SEL], F32, tag="rows")
    RY2 = ps_rows.tile([128, SEL], F32, tag="rows")
    RA7 = ps_rows.tile([128, SEL], F32, tag="rows")
    for (rt, fidx) in ((RX1, 0), (RY1, 1), (RX2, 2), (RY2, 3), (RA7, 4)):
        for c in range(4):
            nc.tensor.transpose(out=rt[:, 128 * c:128 * (c + 1)],
                                in_=ordb[:, c, fidx:fidx + 1].to_broadcast([128, 128]),
                                identity=ident_sb[:])

    # ---------------- phase 8: suppression masks ----------------
    M = []
    for c in range(4):
        f0 = 128 * c
        Wc = SEL - f0
        t1 = sbw.tile([128, SEL], F32, tag="mw")
        nc.vector.tensor_scalar(t1[:, 0:Wc], RX2[:, f0:SEL], ordb[:, c, 2:3],
                                None, OP.min)
        u = sbw.tile([128, SEL], F32, tag="mw")
        nc.vector.scalar_tensor_tensor(u[:, 0:Wc], in0=RX1[:, f0:SEL],
                                       scalar=ordb[:, c, 0:1], in1=t1[:, 0:Wc],
                                       op0=OP.max, op1=OP.subtract)
        iw = sbw.tile([128, SEL], F32, tag="mw2")
        nc.scalar.activation(iw[:, 0:Wc], u[:, 0:Wc], AF.Relu, bias=b17[:], scale=-1.7)
        t2 = sbw.tile([128, SEL], F32, tag="mw")
        nc.vector.tensor_scalar(t2[:, 0:Wc], RY2[:, f0:SEL], ordb[:, c, 3:4],
                                None, OP.min)
        vv = sbw.tile([128, SEL], F32, tag="mw")
        nc.vector.scalar_tensor_tensor(vv[:, 0:Wc], in0=RY1[:, f0:SEL],
                                       scalar=ordb[:, c, 1:2], in1=t2[:, 0:Wc],
                                       op0=OP.max, op1=OP.subtract)
        ih = sbw.tile([128, SEL], F32, tag="mw2")
        nc.scalar.activation(ih[:, 0:Wc], vv[:, 0:Wc], AF.Relu, bias=b10[:], scale=-1.0)
        it = sbw.tile([128, SEL], F32, tag="mw")
        nc.vector.tensor_tensor(it[:, 0:Wc], iw[:, 0:Wc], ih[:, 0:Wc], op=OP.mult)
        cm = sbw.tile([128, SEL], F32, tag="mw2")
        nc.vector.scalar_tensor_tensor(cm[:, 0:Wc], in0=RA7[:, f0:SEL],
                                       scalar=ordb[:, c, 4:5], in1=it[:, 0:Wc],
                                       op0=OP.add, op1=OP.is_lt)
        Mc = sb.tile([128, SEL], F32, tag=f"M{c}")
        nc.vector.tensor_tensor(Mc[:, 0:Wc], cm[:, 0:Wc],
                                strict_sb[:, c * SEL + f0:(c + 1) * SEL], op=OP.mult)
        M.append(Mc)

    # ---------------- phase 9: greedy NMS as fixpoint ----------------
    alv = ordb[:, :, 5]
    Ka = sb.tile([128, 4], F32, tag="Ka")
    Kb = sb.tile([128, 4], F32, tag="Kb")
    nc.vector.tensor_copy(Ka[:], alv)
    cur, nxt = Ka, Kb
    kept_hist = []
    for _t in range(ITERS + 1):
        for cj in range(4):
            for ci in range(cj + 1):
                nc.tensor.matmul(
                    out=psmall[:, cj:cj + 1],
                    lhsT=M[ci][:, (cj - ci) * 128:(cj - ci) * 128 + 128],
                    rhs=cur[:, ci:ci + 1],
                    start=(ci == 0), stop=(ci == cj))
        nc.vector.scalar_tensor_tensor(nxt[:], in0=psmall[:, 0:4], scalar=0.0,
                                       in1=alv, op0=OP.is_equal, op1=OP.mult)
        kept_hist.append((cur, nxt))
        cur, nxt = nxt, cur
    klast, kprev = kept_hist[-1][1], kept_hist[-1][0]

    dneq = sb.tile([128, 4], F32)
    nc.vector.tensor_tensor(dneq[:], klast[:], kprev[:], op=OP.not_equal)
    nc.tensor.matmul(out=psmall[0:1, 4:8], lhsT=ones_col[:], rhs=dneq[:],
                     start=True, stop=True)

    # ---------------- phase 10: output positions + scatter ----------------
    nc.tensor.matmul(out=psmall[:, 8:12], lhsT=slow_sb[:], rhs=klast[:],
                     start=True, stop=True)
    nc.tensor.matmul(out=psmall[0:1, 12:16], lhsT=ones_col[:], rhs=klast[:],
                     start=True, stop=True)
    cssb = sb.tile([1, 4], F32)
    nc.vector.tensor_copy(cssb[:], psmall[0:1, 12:16])
    incs = sb.tile([1, 4], F32)
    nc.vector.tensor_tensor_scan(incs[:], data0=cssb[:], data1=zeros14[:],
                                 initial=0.0, op0=OP.add, op1=OP.add)
    excs = sb.tile([1, 4], F32)
    nc.vector.tensor_tensor(excs[:], incs[:], cssb[:], op=OP.subtract)
    nc.tensor.matmul(out=psmall[:, 12:16], lhsT=ones_row[:], rhs=excs[:],
                     start=True, stop=True)
    cob = sb.tile([128, 4], F32)
    nc.vector.tensor_copy(cob[:], psmall[:, 12:16])

    dest = sb.tile([128, 4], F32)
    nc.vector.tensor_tensor(dest[:], psmall[:, 8:12], cob[:], op=OP.add)
    mk = sb.tile([128, 4], F32)
    nc.vector.scalar_tensor_tensor(mk[:], in0=dest[:], scalar=float(POST_NMS),
                                   in1=klast[:], op0=OP.is_lt, op1=OP.mult)
    nc.vector.scalar_tensor_tensor(dest[:], in0=dest[:], scalar=float(POST_NMS),
                                   in1=mk[:], op0=OP.subtract, op1=OP.mult)
    nc.vector.tensor_scalar(dest[:], dest[:], float(POST_NMS), None, OP.add)
    dri = sb.tile([128, 4], I32)
    nc.vector.tensor_copy(dri[:], dest[:])

    zer = sb.tile([1, 5 * (POST_NMS + 1)], F32)
    nc.vector.memset(zer[:], 0.0)
    i_zero = nc.sync.dma_start(out=bass.AP(out_t, 0, [[5, POST_NMS + 1], [1, 5]]),
                               in_=zer[:])
    i_sc3 = nc.gpsimd.indirect_dma_start(
        out=bass.AP(out_t, 0, [[5, POST_NMS + 1], [1, 5]]),
        out_offset=bass.IndirectOffsetOnAxis(ap=dri[:], axis=0),
        in_=ordb[:, :, 0:4], in_offset=None,
        element_offset=1)
    add_dep_helper(i_sc3.ins, i_zero.ins, reason="out scatter after zero-fill")

    # ---------------- flags ----------------
    fl = sb.tile([1, 8], F32)
    nc.vector.memset(fl[:], 0.0)
    nc.vector.tensor_copy(fl[:, 0:4], psmall[0:1, 4:8])   # not-converged sums
    nc.vector.tensor_copy(fl[:, 4:5], incs[:, 3:4])       # total kept
    nc.vector.tensor_copy(fl[:, 5:6], psmall[0:1, 17:18])  # num selected
    nc.vector.tensor_copy(fl[:, 6:7], kv[:, 0:1])         # tau
    nc.vector.tensor_copy(fl[:, 7:8], psmall[0:1, 18:19])  # dup/trunc badness
    nc.sync.dma_start(out=flags_t[:], in_=fl[:])

    ctx.close()


# ---------------- host fallback: exact numpy replica of the reference ----------
def _reference_numpy(scores, bbox_deltas, im_info):
    base = _gen_anchors()
    sc = scores[:, A:, :, :].transpose(0, 2, 3, 1).reshape(-1).astype(np.float32)
    sx = (np.arange(W) * FEAT_STRIDE).astype(np.float32)
    sy = (np.arange(H) * FEAT_STRIDE).astype(np.float32)
    gy, gx = np.meshgrid(sy, sx, indexing="ij")
    shifts = np.stack([gx, gy, gx, gy], axis=-1).reshape(-1, 1, 4)
    anchors = (base[None, :, :] + shifts).reshape(-1, 4).astype(np.float32)
    d = bbox_deltas.transpose(0, 2, 3, 1).reshape(-1, 4).astype(np.float32)
    d = np.concatenate([d[:, :2], np.clip(d[:, 2:], -10.0, 10.0)], axis=1)
    aw = anchors[:, 2] - anchors[:, 0] + 1.0
    ah = anchors[:, 3] - anchors[:, 1] + 1.0
    acx = anchors[:, 0] + 0.5 * aw
    acy = anchors[:, 1] + 0.5 * ah
    pcx = d[:, 0] * aw + acx
    pcy = d[:, 1] * ah + acy
    pw = np.exp(d[:, 2]) * aw
    ph = np.exp(d[:, 3]) * ah
    boxes = np.stack([pcx - 0.5 * pw, pcy - 0.5 * ph,
                      pcx + 0.5 * pw, pcy + 0.5 * ph], axis=1).astype(np.float32)
    im_h, im_w, im_s = im_info[0]
    boxes = np.stack([np.clip(boxes[:, 0], 0, im_w - 1),
                      np.clip(boxes[:, 1], 0, im_h - 1),
                      np.clip(boxes[:, 2], 0, im_w - 1),
                      np.clip(boxes[:, 3], 0, im_h - 1)], axis=1).astype(np.float32)
    ws = boxes[:, 2] - boxes[:, 0] + 1.0
    hs = boxes[:, 3] - boxes[:, 1] + 1.0
    valid = (ws >= MIN_SIZE * im_s) & (hs >= MIN_SIZE * im_s)
    scm = np.where(valid, sc, -np.inf).astype(np.float32)
    order = np.argsort(-scm, kind="stable")[:PRE_NMS]
    top_sc = scm[order]
    props = boxes[order]
    x1, y1, x2, y2 = props[:, 0], props[:, 1], props[:, 2], props[:, 3]
    areas = (x2 - x1 + 1.0) * (y2 - y1 + 1.0)
    keep = np.isfinite(top_sc)
    for i in range(PRE_NMS):
        if not keep[i]:
            continue
        xx1 = np.maximum(x1[i], x1[i + 1:])
        yy1 = np.maximum(y1[i], y1[i + 1:])
        xx2 = np.minimum(x2[i], x2[i + 1:])
        yy2 = np.minimum(y2[i], y2[i + 1:])
        iw = np.clip(xx2 - xx1 + 1.0, 0.0, None)
        ih = np.clip(yy2 - yy1 + 1.0, 0.0, None)
        inter = iw * ih
        iou = inter / (areas[i] + areas[i + 1:] - inter)
        keep[i + 1:] &= ~(iou > NMS_THRESH)
    out = np.zeros((POST_NMS, 5), dtype=np.float32)
    kk = np.where(keep)[0][:POST_NMS]
    out[np.arange(len(kk)), 1:] = props[kk]
    return out


_PROGRAM = None


def _get_program():
    global _PROGRAM
    if _PROGRAM is None:
        _PROGRAM = build_program()
    return _PROGRAM


def make_in_map(scores, bbox_deltas, im_info):
    return {
        "scores": np.ascontiguousarray(scores, dtype=np.float32),
        "bbox_deltas": np.ascontiguousarray(bbox_deltas, dtype=np.float32),
        "im_info": np.ascontiguousarray(im_info, dtype=np.float32),
        "recs": _REC,
        "strict": _STRICT,
        "slow": _SLOW,
        "ident": _IDENT,
        "slotid": _SLOTID,
        "kidx": _KIDX,
        "pbase": _PBASE,
    }


def kernel(scores, bbox_deltas, im_info):
    nc = _get_program()
    in_map = make_in_map(scores, bbox_deltas, im_info)
    res = run_bass_kernel_spmd(nc, [in_map] * 8, core_ids=list(range(8)))
    r0 = res.results[0]
    out = np.asarray(r0["out"])[:POST_NMS]
    flags = np.asarray(r0["flags"]).reshape(-1)
    notconv = flags[0:4].sum() > 0
    kept = flags[4]
    nsel = flags[5]
    bad = flags[7]
    if notconv or kept < POST_NMS or nsel > SEL or bad > 0:
        out = _reference_numpy(np.asarray(scores), np.asarray(bbox_deltas),
                               np.asarray(im_info))
    return out.astype(np.float32)
